# revision 1
# baseline (speedup 1.0000x reference)
"""3-layer dense GAT (N=4096, F=512, H=8 heads, D=64) on 8 TRN2 NeuronCores.

Strategy (1D row-parallel):
  - Each core owns LOCAL=512 query rows i. Per layer, each core computes its
    local hext = x_local @ [W | W@a1 | W@a2] (so f1/f2 come out as extra
    matmul columns), exponentiates f2 branches locally, and AllGathers
    [h(bf16) | exp(f2) | exp(alpha*f2)] across the 8 cores.
  - Attention scores: exp(lrelu(f1_i + f2_j)) == max(exp(f1_i)*exp(f2_j),
    exp(a*f1_i)*exp(a*f2_j)) -- exp(max)=max(exp), and each branch is a
    separable product. So P^T tiles [j=128, i=512] need no transcendentals:
    one tensor_scalar (v branch), one fused scalar_tensor_tensor (u branch +
    max), one tensor_tensor (adjacency mask) per tile, all bf16.
  - Matmul: out[d,i] += h_block[j,d|ones].T @ P^T[j,i] accumulated over the
    32 j-chunks; the appended ones column yields the softmax denominator Z
    in PSUM row 64 for free. h' = U/Z, then ELU; the [d,i] orientation is
    exactly the next layer's lhsT (x^T), so no transposes are needed.
"""

import numpy as np
import ml_dtypes

import concourse.bass as bass
import concourse.mybir as mybir
from concourse import bacc, tile, masks
from concourse.bass_utils import run_bass_kernel_spmd

N = 4096
F = 512
D = 64
H = 8
NCORES = 8
LOCAL = N // NCORES          # 512 query rows per core
JC = N // 128                # 32 j-chunks
IC = LOCAL // 128            # 4 local i-chunks
FC = F // 128                # 4 contraction chunks
NL = 3
SLOT = 66                    # per-head cols in stationary buf: 64 h + ones + pad
CH = H * SLOT + 2 * H        # 544: per-j-chunk stride (8 slots + 8 EC + 8 ED)
ALPHA = 0.2
f32 = mybir.dt.float32
bf16 = mybir.dt.bfloat16
BF = ml_dtypes.bfloat16
OP = mybir.AluOpType
AF = mybir.ActivationFunctionType


def build_nc(debug_dumps=False):
    nc = bacc.Bacc(None, target_bir_lowering=False, num_devices=NCORES)

    xT_d = nc.dram_tensor("xT", [F, LOCAL], bf16, kind="ExternalInput")
    maskT_d = nc.dram_tensor("maskT", [N, LOCAL], bf16, kind="ExternalInput")
    wext_d = nc.dram_tensor("wext", [NL, F, H * SLOT], bf16, kind="ExternalInput")
    out_d = nc.dram_tensor("outT", [D, LOCAL], f32, kind="ExternalOutput")
    if debug_dumps:
        dbg_lg = nc.dram_tensor("dbg_lg", [LOCAL, F + 2 * H], bf16, kind="ExternalOutput")
        dbg_gg = nc.dram_tensor("dbg_gg", [N, F + 2 * H], bf16, kind="ExternalOutput")
        dbg_eab = nc.dram_tensor("dbg_eab", [128, H * 2 * LOCAL], bf16, kind="ExternalOutput")
        dbg_xt = nc.dram_tensor("dbg_xt", [128, FC * LOCAL], bf16, kind="ExternalOutput")
        dbg_gs = nc.dram_tensor("dbg_gs", [128, JC * CH], bf16, kind="ExternalOutput")
        dbg_r = nc.dram_tensor("dbg_r", [H, LOCAL], f32, kind="ExternalOutput")
        dbg_z = nc.dram_tensor("dbg_z", [H, LOCAL], f32, kind="ExternalOutput")

    with tile.TileContext(nc) as tc:
        with (
            tc.tile_pool(name="persist", bufs=1) as pp,
            tc.tile_pool(name="ident", bufs=1) as ident_pool,
            tc.tile_pool(name="hc", bufs=3) as hc_pool,
            tc.tile_pool(name="ecd", bufs=2) as ecd_pool,
            tc.tile_pool(name="ea8", bufs=2) as ea8_pool,
            tc.tile_pool(name="vtile", bufs=3) as v_pool,
            tc.tile_pool(name="ptile", bufs=3) as p_pool,
            tc.tile_pool(name="pmtile", bufs=4) as pm_pool,
            tc.tile_pool(name="norm", bufs=2) as nm_pool,
            tc.tile_pool(name="psA", bufs=3, space="PSUM") as psA,
            tc.tile_pool(name="psB", bufs=3, space="PSUM") as psB,
            tc.tile_pool(name="psT", bufs=1, space="PSUM") as psT,
            tc.tile_pool(name="dram", bufs=1, space="DRAM") as dram,
        ):
            # ---- persistent SBUF ----
            XT = pp.tile([128, FC * LOCAL], bf16, tag="XT")        # x^T local
            MASK = pp.tile([128, JC * LOCAL], bf16, tag="MASK")    # mask^T
            WEXT = pp.tile([128, NL * FC * H * SLOT], bf16, tag="WEXT")
            GS = pp.tile([128, JC * CH], bf16, tag="GS")           # gathered stationary
            EAB = pp.tile([128, H * 2 * LOCAL], bf16, tag="EAB")   # exp(f1) bcasts
            F12 = pp.tile([128, IC * H * 2], f32, tag="F12")       # f1,f2 cols local
            F12T = pp.tile([16, IC * 128], f32, tag="F12T")        # transposed
            ECDF = pp.tile([128, JC * 2 * H], f32, tag="ECDF")     # f32 exp(f2) cols
            ACCa = pp.tile([D, LOCAL], f32, tag="ACCa")            # layer-3 head mean
            ACCb = pp.tile([D, LOCAL], f32, tag="ACCb")
            OUTS = pp.tile([D, LOCAL], f32, tag="OUTS")
            IDENT = ident_pool.tile([128, 128], f32)

            # DRAM bounce buffers for the all-gather
            LG = dram.tile([LOCAL, F + 2 * H], bf16, tag="LG")
            GGs = [
                dram.tile(
                    [N, F + 2 * H], bf16, tag=f"GG{l}", addr_space="Shared",
                    name=f"GG{l}",
                )
                for l in range(NL)
            ]
            EDR = dram.tile([2, 16, LOCAL], bf16, tag="EDR")  # exp(f1) bounce

            # views
            X2 = XT[:].rearrange("p (fc i) -> p fc i", i=LOCAL)
            M2 = MASK[:].rearrange("p (c i) -> p c i", i=LOCAL)
            W4 = WEXT[:].rearrange("p (l fc s) -> p l fc s", l=NL, fc=FC)
            GS2 = GS[:].rearrange("p (c s) -> p c s", s=CH)
            ECD3 = ECDF[:].rearrange("p (c s) -> p c s", s=2 * H)
            EA4 = EAB[:].rearrange("p (h t i) -> p h t i", h=H, t=2)

            # ---- one-time loads ----
            nc.sync.dma_start(X2, xT_d[:].rearrange("(fc p) i -> p fc i", p=128))
            nc.sync.dma_start(M2, maskT_d[:].rearrange("(c p) i -> p c i", p=128))
            nc.sync.dma_start(
                W4, wext_d[:].rearrange("l (fc p) s -> p l fc s", p=128)
            )
            masks.make_identity(nc, IDENT[:])
            nc.gpsimd.memset(GS[:], 0.0)
            ones_view = GS2[:, :, 0 : H * SLOT].rearrange(
                "p c (h u) -> p c h u", u=SLOT
            )[:, :, :, D : D + 1]
            nc.gpsimd.memset(ones_view, 1.0)

            for l in range(NL):
                # ---- Phase A: local hext = x_local @ Wext ----
                for ic in range(IC):
                    for h in range(H):
                        ps = psA.tile([128, SLOT], f32, tag="psA")
                        for fc in range(FC):
                            nc.tensor.matmul(
                                ps[:],
                                X2[:, fc, ic * 128 : (ic + 1) * 128],
                                W4[:, l, fc, h * SLOT : (h + 1) * SLOT],
                                start=(fc == 0),
                                stop=(fc == FC - 1),
                            )
                        hcz = hc_pool.tile([128, D], bf16, tag="hc")
                        nc.vector.tensor_copy(hcz[:], ps[:, 0:D])
                        nc.sync.dma_start(
                            LG[ic * 128 : (ic + 1) * 128, h * D : (h + 1) * D],
                            hcz[:],
                        )
                        nc.scalar.copy(
                            F12[:, (ic * H + h) * 2 : (ic * H + h) * 2 + 2],
                            ps[:, D : D + 2],
                        )

                # ---- Phase A2: f1 -> exp rows broadcast across partitions ----
                for ic in range(IC):
                    pt = psT.tile([16, 128], f32, tag="psT")
                    nc.tensor.transpose(
                        pt[:], F12[:, ic * 16 : (ic + 1) * 16], IDENT[:]
                    )
                    nc.vector.tensor_copy(
                        F12T[:, ic * 128 : (ic + 1) * 128], pt[:]
                    )
                ea8 = ea8_pool.tile([16, LOCAL], bf16, tag="ea8")
                eb8 = ea8_pool.tile([16, LOCAL], bf16, tag="eb8")
                nc.scalar.activation(ea8[:], F12T[:], AF.Exp)
                nc.scalar.activation(eb8[:], F12T[:], AF.Exp, scale=ALPHA)
                nc.sync.dma_start(EDR[0], ea8[:])
                nc.sync.dma_start(EDR[1], eb8[:])
                for h in range(H):
                    for t in range(2):
                        nc.sync.dma_start(
                            EA4[:, h, t, :],
                            EDR[t, 2 * h : 2 * h + 1, :].partition_broadcast(128),
                        )

                # ---- Phase A3: local exp(f2), exp(a*f2) -> gather buffer ----
                for ic in range(IC):
                    ec = ecd_pool.tile([128, 2 * H], bf16, tag="ec")
                    ed = ecd_pool.tile([128, 2 * H], bf16, tag="ed")
                    nc.scalar.activation(ec[:], F12[:, ic * 16 : (ic + 1) * 16], AF.Exp)
                    nc.scalar.activation(
                        ed[:], F12[:, ic * 16 : (ic + 1) * 16], AF.Exp, scale=ALPHA
                    )
                    rows = slice(ic * 128, (ic + 1) * 128)
                    nc.sync.dma_start(
                        LG[rows, F : F + H],
                        ec[:].rearrange("p (h t) -> p h t", t=2)[:, :, 1],
                    )
                    nc.sync.dma_start(
                        LG[rows, F + H : F + 2 * H],
                        ed[:].rearrange("p (h t) -> p h t", t=2)[:, :, 1],
                    )

                # ---- Phase B: all-gather and stage into SBUF ----
                GG = GGs[l]
                nc.gpsimd.collective_compute(
                    "AllGather",
                    OP.bypass,
                    replica_groups=[list(range(NCORES))],
                    ins=[LG.opt()],
                    outs=[GG.opt()],
                )
                gsrc = GG[:].rearrange("(c p) s -> p c s", p=128)
                gdst = GS2[:, :, 0 : H * SLOT].rearrange(
                    "p c (h u) -> p c h u", u=SLOT
                )
                for h in range(H):
                    nc.sync.dma_start(
                        gdst[:, :, h, 0:D],
                        gsrc[:, :, h * D : (h + 1) * D],
                    )
                nc.sync.dma_start(
                    GS2[:, :, H * SLOT : H * SLOT + 2 * H],
                    gsrc[:, :, F : F + 2 * H],
                )
                # f32 copies of exp(f2)/exp(a*f2) for tensor_scalar operands
                nc.vector.tensor_copy(
                    ECD3[:, :, :], GS2[:, :, H * SLOT : H * SLOT + 2 * H]
                )
                if debug_dumps and l == 0:
                    nc.sync.dma_start(dbg_lg[:], LG[:])
                    nc.sync.dma_start(dbg_gg[:], GG[:])
                    nc.sync.dma_start(dbg_eab[:], EAB[:])
                    nc.sync.dma_start(dbg_gs[:], GS[:])

                # ---- Phase C/D: attention per head ----
                for h in range(H):
                    pb = psB.tile([SLOT - 1, LOCAL], f32, tag="psB")
                    for jc in range(JC):
                        ec_col = ECD3[:, jc, h : h + 1]
                        ed_col = ECD3[:, jc, H + h : H + h + 1]
                        v = v_pool.tile([128, LOCAL], bf16, tag="v")
                        nc.vector.tensor_scalar(
                            v[:], EA4[:, h, 1, :], ed_col, None, OP.mult
                        )
                        p = p_pool.tile([128, LOCAL], bf16, tag="p")
                        nc.vector.scalar_tensor_tensor(
                            p[:], EA4[:, h, 0, :], ec_col, v[:], OP.mult, OP.max
                        )
                        pm = pm_pool.tile([128, LOCAL], bf16, tag="pm")
                        eng = nc.vector if (jc % 16) < 3 else nc.gpsimd
                        eng.tensor_tensor(pm[:], p[:], M2[:, jc, :], OP.mult)
                        nc.tensor.matmul(
                            pb[:],
                            GS2[:, jc, h * SLOT : h * SLOT + SLOT - 1],
                            pm[:],
                            start=(jc == 0),
                            stop=(jc == JC - 1),
                        )

                    # normalize by Z (PSUM row 64) and apply ELU
                    # 1/Z: DMA-reshape Z [1,512] -> [128,4] so the iterative
                    # divide runs 4-deep across 128 lanes, then reshape back.
                    zrow1 = nm_pool.tile([1, LOCAL], f32, tag="zrow1")
                    nc.vector.tensor_copy(zrow1[:], pb[D : D + 1, :])
                    zz4 = nm_pool.tile([128, LOCAL // 128], f32, tag="zz4")
                    nc.sync.dma_start(zz4[:], zrow1[:])
                    rz4 = nm_pool.tile([128, LOCAL // 128], f32, tag="rz4")
                    nc.vector.reciprocal(rz4[:], zz4[:])
                    r1 = nm_pool.tile([1, LOCAL], f32, tag="r1")
                    nc.sync.dma_start(r1[:], rz4[:])
                    if debug_dumps and l == 0:
                        zrow = nm_pool.tile([1, LOCAL], f32, tag="zrow")
                        nc.vector.tensor_copy(zrow[:], pb[D : D + 1, :])
                        nc.sync.dma_start(dbg_z[h : h + 1, :], zrow[:])
                        nc.sync.dma_start(dbg_r[h : h + 1, :], r1[:])
                    rb = nm_pool.tile([D, LOCAL], f32, tag="rb")
                    nc.gpsimd.partition_broadcast(rb[:], r1[:], channels=D)
                    y = nm_pool.tile([D, LOCAL], f32, tag="y")
                    nc.vector.tensor_tensor(y[:], pb[0:D, :], rb[:], OP.mult)
                    ee = nm_pool.tile([D, LOCAL], f32, tag="ee")
                    nc.scalar.activation(ee[:], y[:], AF.Exp)
                    ry = nm_pool.tile([D, LOCAL], f32, tag="ry")
                    nc.scalar.activation(ry[:], y[:], AF.Relu)
                    z1 = nm_pool.tile([D, LOCAL], f32, tag="z1")
                    nc.vector.scalar_tensor_tensor(
                        z1[:], ee[:], 1.0, ry[:], OP.min, OP.add
                    )
                    if l < NL - 1:
                        poff = (h % 2) * D
                        dst = X2[poff : poff + D, h // 2, :]
                        nc.vector.tensor_scalar(dst, z1[:], 1.0, None, OP.subtract)
                    else:
                        ey = nm_pool.tile([D, LOCAL], f32, tag="ey")
                        nc.vector.tensor_scalar(ey[:], z1[:], 1.0, None, OP.subtract)
                        if h == 0:
                            nc.vector.tensor_copy(ACCa[:], ey[:])
                        else:
                            src, dst_acc = (
                                (ACCa, ACCb) if h % 2 == 1 else (ACCb, ACCa)
                            )
                            nc.vector.tensor_tensor(
                                dst_acc[:], src[:], ey[:], OP.add
                            )

                if debug_dumps and l == 0:
                    nc.sync.dma_start(dbg_xt[:], XT[:])

            # ---- final: mean over heads, ELU, write out ----
            fin = ACCb if (H - 1) % 2 == 1 else ACCa
            m1 = nm_pool.tile([D, LOCAL], f32, tag="m1")
            nc.vector.tensor_scalar(m1[:], fin[:], 1.0 / H, None, OP.mult)
            e2 = nm_pool.tile([D, LOCAL], f32, tag="e2")
            nc.scalar.activation(e2[:], m1[:], AF.Exp)
            r2 = nm_pool.tile([D, LOCAL], f32, tag="r2")
            nc.scalar.activation(r2[:], m1[:], AF.Relu)
            nc.vector.scalar_tensor_tensor(
                OUTS[:], e2[:], 1.0, r2[:], OP.min, OP.add
            )
            nc.vector.tensor_scalar(OUTS[:], OUTS[:], 1.0, None, OP.subtract)
            nc.sync.dma_start(out_d[:], OUTS[:])

    nc.compile()
    return nc


def _prep_inputs(inputs):
    x = np.asarray(inputs["x"], np.float32)
    adj = np.asarray(inputs["adj"])
    Ws = [np.asarray(inputs[k], np.float32) for k in ("W1", "W2", "W3")]
    a1s = [np.asarray(inputs[k], np.float32) for k in ("a1_1", "a1_2", "a1_3")]
    a2s = [np.asarray(inputs[k], np.float32) for k in ("a2_1", "a2_2", "a2_3")]

    wext = np.zeros((NL, F, H * SLOT), np.float32)
    for l in range(NL):
        for h in range(H):
            wext[l, :, h * SLOT : h * SLOT + D] = Ws[l][h]
            wext[l, :, h * SLOT + D] = Ws[l][h] @ a1s[l][h]
            wext[l, :, h * SLOT + D + 1] = Ws[l][h] @ a2s[l][h]
    wext_bf = np.ascontiguousarray(wext.astype(BF))

    mask = adj > 0
    in_maps = []
    for c in range(NCORES):
        rows = slice(c * LOCAL, (c + 1) * LOCAL)
        in_maps.append(
            {
                "xT": np.ascontiguousarray(x[rows].T).astype(BF),
                "maskT": np.ascontiguousarray(mask[rows].T).astype(BF),
                "wext": wext_bf,
            }
        )
    return in_maps


_CACHE = {}


def _run(inputs, trace=False):
    in_maps = _prep_inputs(inputs)
    if "nc" not in _CACHE:
        _CACHE["nc"] = build_nc()
    res = run_bass_kernel_spmd(
        _CACHE["nc"], in_maps, list(range(NCORES)), trace=trace
    )
    outs = [r["outT"] for r in res.results]
    out = np.concatenate([np.asarray(o, np.float32).T for o in outs], axis=0)
    return out, res


def kernel(**inputs) -> np.ndarray:
    out, _ = _run(inputs, trace=False)
    return out.astype(np.float32)



# revision 3
# speedup vs baseline: 1.1383x; 1.1383x over previous
"""3-layer dense GAT (N=4096, F=512, H=8 heads, D=64) on 8 TRN2 NeuronCores.

Strategy (1D row-parallel, exp(f2)-folded attention):
  - Each core owns LOCAL=512 query rows i. Per layer, each core computes its
    local hext = x_local @ [W | W@a1 | W@a2], so f1/f2 come out as extra
    matmul columns.
  - Key identity: with ec_j = exp(f2_j), r_j = exp((a-1) f2_j),
      max(exp(f1_i)exp(f2_j), exp(a f1_i)exp(a f2_j))
        = ec_j * max(exp(f1_i), exp(a f1_i) * r_j).
    So ec is folded into the stationary matrix (ec*h, and ec replaces the
    ones column so the softmax denominator Z still falls out of the same
    matmul), and the per-tile work drops to ONE scalar_tensor_tensor
    (p = max(EA, EB*r), per-partition scalar r) + ONE tensor_tensor mask
    multiply (merged over j-chunk pairs, split across Vector and GpSimd).
  - The scaling ec*h runs on the otherwise-idle Scalar engine as
    activation-Copy with a per-partition scale during PSUM extraction.
  - AllGather is issued per head (8 smaller collectives) so head 0's
    attention overlaps the remaining gathers and staging DMAs.
  - Matmul: out[d,i] += GS_h[j, 0:65].T @ pm[j,i] accumulated over 32
    j-chunks; column 64 of GS_h is ec, so PSUM row 64 is Z. h' = U/Z, ELU;
    the [d,i] orientation is the next layer's lhsT, so no transposes.
"""

import numpy as np
import ml_dtypes

import concourse.bass as bass
import concourse.mybir as mybir
from concourse import bacc, tile, masks
from concourse.bass_utils import run_bass_kernel_spmd

N = 4096
F = 512
D = 64
H = 8
NCORES = 8
LOCAL = N // NCORES          # 512 query rows per core
JC = N // 128                # 32 j-chunks
IC = LOCAL // 128            # 4 local i-chunks
FC = F // 128                # 4 contraction chunks
NL = 3
SLOT = 66                    # per-head cols: 64 ec*h + ec + r
CH = H * SLOT                # 528: per-j-chunk stride in GS
ALPHA = 0.2
f32 = mybir.dt.float32
bf16 = mybir.dt.bfloat16
BF = ml_dtypes.bfloat16
OP = mybir.AluOpType
AF = mybir.ActivationFunctionType


def build_nc():
    nc = bacc.Bacc(None, target_bir_lowering=False, num_devices=NCORES)

    xT_d = nc.dram_tensor("xT", [F, LOCAL], bf16, kind="ExternalInput")
    maskT_d = nc.dram_tensor("maskT", [N, LOCAL], bf16, kind="ExternalInput")
    wext_d = nc.dram_tensor("wext", [NL, F, H * SLOT], bf16, kind="ExternalInput")
    out_d = nc.dram_tensor("outT", [D, LOCAL], f32, kind="ExternalOutput")

    with tile.TileContext(nc) as tc:
        with (
            tc.tile_pool(name="persist", bufs=1) as pp,
            tc.tile_pool(name="ident", bufs=1) as ident_pool,
            tc.tile_pool(name="hc", bufs=4) as hc_pool,
            tc.tile_pool(name="ecd", bufs=3) as ecd_pool,
            tc.tile_pool(name="ea8", bufs=2) as ea8_pool,
            tc.tile_pool(name="ptile", bufs=3) as p_pool,
            tc.tile_pool(name="pmtile", bufs=4) as pm_pool,
            tc.tile_pool(name="norm", bufs=2) as nm_pool,
            tc.tile_pool(name="psA", bufs=2, space="PSUM") as psA,
            tc.tile_pool(name="psB", bufs=2, space="PSUM") as psB,
            tc.tile_pool(name="psT", bufs=1, space="PSUM") as psT,
            tc.tile_pool(name="dram", bufs=1, space="DRAM") as dram,
        ):
            # ---- persistent SBUF ----
            XT = pp.tile([128, FC * LOCAL], bf16, tag="XT")        # x^T local
            MASK = pp.tile([128, JC * LOCAL], bf16, tag="MASK")    # mask^T
            WEXT = pp.tile([128, NL * FC * H * SLOT], bf16, tag="WEXT")
            GS = pp.tile([128, JC * CH], bf16, tag="GS")           # gathered stationary
            EAB = pp.tile([128, H * 2 * LOCAL], bf16, tag="EAB")   # exp(f1) bcasts
            F12 = pp.tile([128, IC * H * 2], f32, tag="F12")       # f1,f2 cols local
            F12T = pp.tile([16, IC * 128], f32, tag="F12T")        # transposed
            RF = pp.tile([128, H * JC], f32, tag="RF")             # f32 r cols (h-major)
            ACCa = pp.tile([D, LOCAL], f32, tag="ACCa")            # layer-3 head mean
            ACCb = pp.tile([D, LOCAL], f32, tag="ACCb")
            OUTS = pp.tile([D, LOCAL], f32, tag="OUTS")
            IDENT = ident_pool.tile([128, 128], f32)

            # DRAM bounce buffers: per-head local slabs + gathered slabs
            LGs = [
                dram.tile([LOCAL, SLOT], bf16, tag=f"LG{h}", name=f"LG{h}")
                for h in range(H)
            ]
            GGs = [
                [
                    dram.tile(
                        [N, SLOT], bf16, tag=f"GG{l}_{h}", addr_space="Shared",
                        name=f"GG{l}_{h}",
                    )
                    for h in range(H)
                ]
                for l in range(NL)
            ]
            EDR = dram.tile([2, 16, LOCAL], bf16, tag="EDR")  # exp(f1) bounce

            # views
            X2 = XT[:].rearrange("p (fc i) -> p fc i", i=LOCAL)
            M2 = MASK[:].rearrange("p (c i) -> p c i", i=LOCAL)
            W4 = WEXT[:].rearrange("p (l fc s) -> p l fc s", l=NL, fc=FC)
            GS2 = GS[:].rearrange("p (c s) -> p c s", s=CH)
            RF2 = RF[:].rearrange("p (h c) -> p h c", h=H)
            EA4 = EAB[:].rearrange("p (h t i) -> p h t i", h=H, t=2)

            # ---- one-time loads ----
            nc.sync.dma_start(X2, xT_d[:].rearrange("(fc p) i -> p fc i", p=128))
            nc.sync.dma_start(M2, maskT_d[:].rearrange("(c p) i -> p c i", p=128))
            nc.sync.dma_start(
                W4, wext_d[:].rearrange("l (fc p) s -> p l fc s", p=128)
            )
            masks.make_identity(nc, IDENT[:])

            for l in range(NL):
                # ---- Phase A: local hext = x_local @ Wext, 4 heads/group ----
                for ic in range(IC):
                    for g in range(2):
                        ps = psA.tile([128, 4 * SLOT], f32, tag="psA")
                        for fc in range(FC):
                            nc.tensor.matmul(
                                ps[:],
                                X2[:, fc, ic * 128 : (ic + 1) * 128],
                                W4[:, l, fc, g * 4 * SLOT : (g + 1) * 4 * SLOT],
                                start=(fc == 0),
                                stop=(fc == FC - 1),
                            )
                        psv = ps[:].rearrange("p (h s) -> p h s", s=SLOT)
                        # f1/f2 -> F12 (same layout as before: (ic h) 2)
                        nc.scalar.copy(
                            F12[:, ic * 16 + g * 8 : ic * 16 + (g + 1) * 8]
                            .rearrange("p (h t) -> p h t", t=2),
                            psv[:, :, D : D + 2],
                        )
                        # ec = exp(f2), r = exp((a-1) f2), per-partition f32
                        ecf = ecd_pool.tile([128, 4], f32, tag="ecf")
                        nc.scalar.activation(ecf[:], psv[:, :, D + 1], AF.Exp)
                        rf = ecd_pool.tile([128, 4], f32, tag="rf")
                        nc.scalar.activation(
                            rf[:], psv[:, :, D + 1], AF.Exp, scale=ALPHA - 1.0
                        )
                        # casts for the gather slab
                        ecb = ecd_pool.tile([128, 4], bf16, tag="ecb")
                        nc.vector.tensor_copy(ecb[:], ecf[:])
                        rb = ecd_pool.tile([128, 4], bf16, tag="rb")
                        nc.vector.tensor_copy(rb[:], rf[:])
                        rows = slice(ic * 128, (ic + 1) * 128)
                        for hh in range(4):
                            h = g * 4 + hh
                            # ec * h on the scalar engine (PSUM src)
                            sh = hc_pool.tile([128, D], bf16, tag="sh")
                            nc.scalar.activation(
                                sh[:],
                                psv[:, hh, 0:D],
                                AF.Copy,
                                scale=ecf[:, hh : hh + 1],
                            )
                            nc.sync.dma_start(LGs[h][rows, 0:D], sh[:])
                            nc.sync.dma_start(
                                LGs[h][rows, D : D + 1], ecb[:, hh : hh + 1]
                            )
                            nc.sync.dma_start(
                                LGs[h][rows, D + 1 : D + 2], rb[:, hh : hh + 1]
                            )

                # ---- Phase A2: f1 -> exp rows broadcast across partitions ----
                for ic in range(IC):
                    pt = psT.tile([16, 128], f32, tag="psT")
                    nc.tensor.transpose(
                        pt[:], F12[:, ic * 16 : (ic + 1) * 16], IDENT[:]
                    )
                    nc.vector.tensor_copy(
                        F12T[:, ic * 128 : (ic + 1) * 128], pt[:]
                    )
                ea8 = ea8_pool.tile([16, LOCAL], bf16, tag="ea8")
                eb8 = ea8_pool.tile([16, LOCAL], bf16, tag="eb8")
                nc.scalar.activation(ea8[:], F12T[:], AF.Exp)
                nc.scalar.activation(eb8[:], F12T[:], AF.Exp, scale=ALPHA)
                nc.sync.dma_start(EDR[0], ea8[:])
                nc.sync.dma_start(EDR[1], eb8[:])
                for h in range(H):
                    for t in range(2):
                        nc.sync.dma_start(
                            EA4[:, h, t, :],
                            EDR[t, 2 * h : 2 * h + 1, :].partition_broadcast(128),
                        )

                # ---- Phase B: per-head all-gather + staging ----
                for h in range(H):
                    GG = GGs[l][h]
                    nc.gpsimd.collective_compute(
                        "AllGather",
                        OP.bypass,
                        replica_groups=[list(range(NCORES))],
                        ins=[LGs[h].opt()],
                        outs=[GG.opt()],
                    )
                    gsrc = GG[:].rearrange("(c p) s -> p c s", p=128)
                    nc.sync.dma_start(
                        GS2[:, :, h * SLOT : (h + 1) * SLOT], gsrc
                    )
                    # f32 copy of r for the STT per-partition scalar
                    nc.vector.tensor_copy(
                        RF2[:, h, :], GS2[:, :, h * SLOT + D + 1]
                    )

                # ---- Phase C/D: attention per head ----
                for h in range(H):
                    pb = psB.tile([SLOT - 1, LOCAL], f32, tag="psB")
                    for jc2 in range(0, JC, 2):
                        p2 = p_pool.tile([128, 2 * LOCAL], bf16, tag="p2")
                        for k in range(2):
                            jc = jc2 + k
                            nc.vector.scalar_tensor_tensor(
                                p2[:, k * LOCAL : (k + 1) * LOCAL],
                                EA4[:, h, 1, :],
                                RF2[:, h, jc : jc + 1],
                                EA4[:, h, 0, :],
                                OP.mult,
                                OP.max,
                            )
                        pm2 = pm_pool.tile([128, 2 * LOCAL], bf16, tag="pm2")
                        eng = nc.vector if (jc2 % 8) == 0 else nc.gpsimd
                        eng.tensor_tensor(
                            pm2[:],
                            p2[:],
                            MASK[:, jc2 * LOCAL : (jc2 + 2) * LOCAL],
                            OP.mult,
                        )
                        for k in range(2):
                            jc = jc2 + k
                            nc.tensor.matmul(
                                pb[:],
                                GS2[:, jc, h * SLOT : h * SLOT + SLOT - 1],
                                pm2[:, k * LOCAL : (k + 1) * LOCAL],
                                start=(jc == 0),
                                stop=(jc == JC - 1),
                            )

                    # normalize by Z (PSUM row 64) and apply ELU
                    # 1/Z: DMA-reshape Z [1,512] -> [128,4] so the iterative
                    # divide runs 4-deep across 128 lanes, then reshape back.
                    zrow1 = nm_pool.tile([1, LOCAL], f32, tag="zrow1")
                    nc.vector.tensor_copy(zrow1[:], pb[D : D + 1, :])
                    zz4 = nm_pool.tile([128, LOCAL // 128], f32, tag="zz4")
                    nc.sync.dma_start(zz4[:], zrow1[:])
                    rz4 = nm_pool.tile([128, LOCAL // 128], f32, tag="rz4")
                    nc.vector.reciprocal(rz4[:], zz4[:])
                    r1 = nm_pool.tile([1, LOCAL], f32, tag="r1")
                    nc.sync.dma_start(r1[:], rz4[:])
                    rb2 = nm_pool.tile([D, LOCAL], f32, tag="rb2")
                    nc.gpsimd.partition_broadcast(rb2[:], r1[:], channels=D)
                    y = nm_pool.tile([D, LOCAL], f32, tag="y")
                    nc.vector.tensor_tensor(y[:], pb[0:D, :], rb2[:], OP.mult)
                    ee = nm_pool.tile([D, LOCAL], f32, tag="ee")
                    nc.scalar.activation(ee[:], y[:], AF.Exp)
                    ry = nm_pool.tile([D, LOCAL], f32, tag="ry")
                    nc.scalar.activation(ry[:], y[:], AF.Relu)
                    z1 = nm_pool.tile([D, LOCAL], f32, tag="z1")
                    nc.vector.scalar_tensor_tensor(
                        z1[:], ee[:], 1.0, ry[:], OP.min, OP.add
                    )
                    if l < NL - 1:
                        poff = (h % 2) * D
                        dst = X2[poff : poff + D, h // 2, :]
                        nc.vector.tensor_scalar(dst, z1[:], 1.0, None, OP.subtract)
                    else:
                        ey = nm_pool.tile([D, LOCAL], f32, tag="ey")
                        nc.vector.tensor_scalar(ey[:], z1[:], 1.0, None, OP.subtract)
                        if h == 0:
                            nc.vector.tensor_copy(ACCa[:], ey[:])
                        else:
                            src, dst_acc = (
                                (ACCa, ACCb) if h % 2 == 1 else (ACCb, ACCa)
                            )
                            nc.vector.tensor_tensor(
                                dst_acc[:], src[:], ey[:], OP.add
                            )

            # ---- final: mean over heads, ELU, write out ----
            fin = ACCb if (H - 1) % 2 == 1 else ACCa
            m1 = nm_pool.tile([D, LOCAL], f32, tag="m1")
            nc.vector.tensor_scalar(m1[:], fin[:], 1.0 / H, None, OP.mult)
            e2 = nm_pool.tile([D, LOCAL], f32, tag="e2")
            nc.scalar.activation(e2[:], m1[:], AF.Exp)
            r2 = nm_pool.tile([D, LOCAL], f32, tag="r2")
            nc.scalar.activation(r2[:], m1[:], AF.Relu)
            nc.vector.scalar_tensor_tensor(
                OUTS[:], e2[:], 1.0, r2[:], OP.min, OP.add
            )
            nc.vector.tensor_scalar(OUTS[:], OUTS[:], 1.0, None, OP.subtract)
            nc.sync.dma_start(out_d[:], OUTS[:])

    nc.compile()
    return nc


def _prep_inputs(inputs):
    x = np.asarray(inputs["x"], np.float32)
    adj = np.asarray(inputs["adj"])
    Ws = [np.asarray(inputs[k], np.float32) for k in ("W1", "W2", "W3")]
    a1s = [np.asarray(inputs[k], np.float32) for k in ("a1_1", "a1_2", "a1_3")]
    a2s = [np.asarray(inputs[k], np.float32) for k in ("a2_1", "a2_2", "a2_3")]

    wext = np.zeros((NL, F, H * SLOT), np.float32)
    for l in range(NL):
        for h in range(H):
            wext[l, :, h * SLOT : h * SLOT + D] = Ws[l][h]
            wext[l, :, h * SLOT + D] = Ws[l][h] @ a1s[l][h]
            wext[l, :, h * SLOT + D + 1] = Ws[l][h] @ a2s[l][h]
    wext_bf = np.ascontiguousarray(wext.astype(BF))

    mask = adj > 0
    in_maps = []
    for c in range(NCORES):
        rows = slice(c * LOCAL, (c + 1) * LOCAL)
        in_maps.append(
            {
                "xT": np.ascontiguousarray(x[rows].T).astype(BF),
                "maskT": np.ascontiguousarray(mask[rows].T).astype(BF),
                "wext": wext_bf,
            }
        )
    return in_maps


_CACHE = {}


def _run(inputs, trace=False):
    in_maps = _prep_inputs(inputs)
    if "nc" not in _CACHE:
        _CACHE["nc"] = build_nc()
    res = run_bass_kernel_spmd(
        _CACHE["nc"], in_maps, list(range(NCORES)), trace=trace
    )
    outs = [r["outT"] for r in res.results]
    out = np.concatenate([np.asarray(o, np.float32).T for o in outs], axis=0)
    return out, res


def kernel(**inputs) -> np.ndarray:
    out, _ = _run(inputs, trace=False)
    return out.astype(np.float32)


# revision 11
# speedup vs baseline: 1.2763x; 1.1213x over previous
"""3-layer dense GAT (N=4096, F=512, H=8 heads, D=64) on 8 TRN2 NeuronCores.

Strategy (1D row-parallel, exp(f2)-folded attention):
  - Each core owns LOCAL=512 query rows i. Per layer, each core computes its
    local hext = x_local @ [W | W@a1 | W@a2], so f1/f2 come out as extra
    matmul columns.
  - Key identity: with ec_j = exp(f2_j), r_j = exp((a-1) f2_j),
      max(exp(f1_i)exp(f2_j), exp(a f1_i)exp(a f2_j))
        = ec_j * max(exp(f1_i), exp(a f1_i) * r_j).
    So ec is folded into the stationary matrix (ec*h, and ec replaces the
    ones column so the softmax denominator Z still falls out of the same
    matmul), and the per-tile work drops to ONE scalar_tensor_tensor
    (p = max(EA, EB*r), per-partition scalar r) + ONE tensor_tensor mask
    multiply (merged over j-chunk pairs, split across Vector and GpSimd).
  - The scaling ec*h runs on the otherwise-idle Scalar engine as
    activation-Copy with a per-partition scale during PSUM extraction.
  - AllGather is issued per head (8 smaller collectives) so head 0's
    attention overlaps the remaining gathers and staging DMAs.
  - Matmul: out[d,i] += GS_h[j, 0:65].T @ pm[j,i] accumulated over 32
    j-chunks; column 64 of GS_h is ec, so PSUM row 64 is Z. h' = U/Z, ELU;
    the [d,i] orientation is the next layer's lhsT, so no transposes.
"""

import numpy as np
import ml_dtypes

import concourse.bass as bass
import concourse.mybir as mybir
from concourse import bacc, tile, masks
from concourse.bass_utils import run_bass_kernel_spmd

N = 4096
F = 512
D = 64
H = 8
NCORES = 8
LOCAL = N // NCORES          # 512 query rows per core
JC = N // 128                # 32 j-chunks
IC = LOCAL // 128            # 4 local i-chunks
FC = F // 128                # 4 contraction chunks
NL = 3
SLOT = 66                    # per-head cols: 64 ec*h + ec + r
CH = H * SLOT                # 528: per-j-chunk stride in GS
ALPHA = 0.2
f32 = mybir.dt.float32
bf16 = mybir.dt.bfloat16
BF = ml_dtypes.bfloat16
OP = mybir.AluOpType
AF = mybir.ActivationFunctionType


def build_nc():
    nc = bacc.Bacc(None, target_bir_lowering=False, num_devices=NCORES)

    xT_d = nc.dram_tensor("xT", [F, LOCAL], bf16, kind="ExternalInput")
    maskT_d = nc.dram_tensor("maskT", [N, LOCAL], bf16, kind="ExternalInput")
    wext_d = nc.dram_tensor("wext", [NL, F, H * SLOT], bf16, kind="ExternalInput")
    out_d = nc.dram_tensor("outT", [D, LOCAL], f32, kind="ExternalOutput")

    with tile.TileContext(nc) as tc:
        with (
            tc.tile_pool(name="persist", bufs=1) as pp,
            tc.tile_pool(name="ident", bufs=1) as ident_pool,
            tc.tile_pool(name="hc", bufs=4) as hc_pool,
            tc.tile_pool(name="ecd", bufs=3) as ecd_pool,
            tc.tile_pool(name="ea8", bufs=2) as ea8_pool,
            tc.tile_pool(name="ptile", bufs=3) as p_pool,
            tc.tile_pool(name="pmtile", bufs=4) as pm_pool,
            tc.tile_pool(name="norm", bufs=2) as nm_pool,
            tc.tile_pool(name="psA", bufs=2, space="PSUM") as psA,
            tc.tile_pool(name="psB", bufs=2, space="PSUM") as psB,
            tc.tile_pool(name="psT", bufs=1, space="PSUM") as psT,
            tc.tile_pool(name="dram", bufs=1, space="DRAM") as dram,
        ):
            # ---- persistent SBUF ----
            XT = pp.tile([128, FC * LOCAL], bf16, tag="XT")        # x^T local
            MASK = pp.tile([128, JC * LOCAL], bf16, tag="MASK")    # mask^T
            WEXT = pp.tile([128, NL * FC * H * SLOT], bf16, tag="WEXT")
            GS = pp.tile([128, JC * CH], bf16, tag="GS")           # gathered stationary
            EAB = pp.tile([128, H * 2 * LOCAL], bf16, tag="EAB")   # exp(f1) bcasts
            F12 = pp.tile([128, IC * H * 2], f32, tag="F12")       # f1,f2 cols local
            F12T = pp.tile([16, IC * 128], f32, tag="F12T")        # transposed
            RF = pp.tile([128, H * JC], f32, tag="RF")             # f32 r cols (h-major)
            ACCa = pp.tile([D, LOCAL], f32, tag="ACCa")            # layer-3 head mean
            ACCb = pp.tile([D, LOCAL], f32, tag="ACCb")
            OUTS = pp.tile([D, LOCAL], f32, tag="OUTS")
            IDENT = ident_pool.tile([128, 128], f32)

            # DRAM bounce buffers: per-head local slabs + gathered slabs
            LGs = [
                dram.tile([LOCAL, SLOT], bf16, tag=f"LG{h}", name=f"LG{h}")
                for h in range(H)
            ]
            GGs = [
                [
                    dram.tile(
                        [N, SLOT], bf16, tag=f"GG{l}_{h}", addr_space="Shared",
                        name=f"GG{l}_{h}",
                    )
                    for h in range(H)
                ]
                for l in range(NL)
            ]
            EDR = dram.tile([2, 16, LOCAL], bf16, tag="EDR")  # exp(f1) bounce

            # views
            X2 = XT[:].rearrange("p (fc i) -> p fc i", i=LOCAL)
            M2 = MASK[:].rearrange("p (c i) -> p c i", i=LOCAL)
            W4 = WEXT[:].rearrange("p (l fc s) -> p l fc s", l=NL, fc=FC)
            GS2 = GS[:].rearrange("p (c s) -> p c s", s=CH)
            RF2 = RF[:].rearrange("p (h c) -> p h c", h=H)
            EA4 = EAB[:].rearrange("p (h t i) -> p h t i", h=H, t=2)

            # ---- one-time loads ----
            nc.sync.dma_start(X2, xT_d[:].rearrange("(fc p) i -> p fc i", p=128))
            nc.sync.dma_start(M2, maskT_d[:].rearrange("(c p) i -> p c i", p=128))
            nc.sync.dma_start(
                W4, wext_d[:].rearrange("l (fc p) s -> p l fc s", p=128)
            )
            masks.make_identity(nc, IDENT[:])
            NEG1 = pp.tile([128, 1], f32, tag="NEG1")
            nc.gpsimd.memset(NEG1[:], -1.0)

            for l in range(NL):
                # ---- Phase A: local hext = x_local @ Wext, 4 heads/group ----
                for ic in range(IC):
                    for g in range(2):
                        ps = psA.tile([128, 4 * SLOT], f32, tag="psA")
                        for fc in range(FC):
                            nc.tensor.matmul(
                                ps[:],
                                X2[:, fc, ic * 128 : (ic + 1) * 128],
                                W4[:, l, fc, g * 4 * SLOT : (g + 1) * 4 * SLOT],
                                start=(fc == 0),
                                stop=(fc == FC - 1),
                            )
                        psv = ps[:].rearrange("p (h s) -> p h s", s=SLOT)
                        # f1/f2 -> F12 (same layout as before: (ic h) 2)
                        nc.scalar.copy(
                            F12[:, ic * 16 + g * 8 : ic * 16 + (g + 1) * 8]
                            .rearrange("p (h t) -> p h t", t=2),
                            psv[:, :, D : D + 2],
                        )
                        # ec = exp(f2), r = exp((a-1) f2), per-partition f32
                        ecf = ecd_pool.tile([128, 4], f32, tag="ecf")
                        nc.scalar.activation(ecf[:], psv[:, :, D + 1], AF.Exp)
                        rf = ecd_pool.tile([128, 4], f32, tag="rf")
                        nc.scalar.activation(
                            rf[:], psv[:, :, D + 1], AF.Exp, scale=ALPHA - 1.0
                        )
                        # assemble [ec*h | ec | r] for 4 heads in SBUF, then
                        # one DMA per head slab (avoids a tiny-DMA flood)
                        sg4 = hc_pool.tile([128, 4 * SLOT], bf16, tag="sg4")
                        sg4v = sg4[:].rearrange("p (h s) -> p h s", s=SLOT)
                        for hh in range(4):
                            nc.scalar.activation(
                                sg4v[:, hh, 0:D],
                                psv[:, hh, 0:D],
                                AF.Copy,
                                scale=ecf[:, hh : hh + 1],
                            )
                        nc.vector.tensor_copy(sg4v[:, :, D], ecf[:])
                        nc.vector.tensor_copy(sg4v[:, :, D + 1], rf[:])
                        rows = slice(ic * 128, (ic + 1) * 128)
                        for hh in range(4):
                            h = g * 4 + hh
                            nc.sync.dma_start(LGs[h][rows, :], sg4v[:, hh, :])

                # ---- Phase A2: f1 -> exp rows broadcast across partitions ----
                for ic in range(IC):
                    pt = psT.tile([16, 128], f32, tag="psT")
                    nc.tensor.transpose(
                        pt[:], F12[:, ic * 16 : (ic + 1) * 16], IDENT[:]
                    )
                    nc.vector.tensor_copy(
                        F12T[:, ic * 128 : (ic + 1) * 128], pt[:]
                    )
                ea8 = ea8_pool.tile([16, LOCAL], bf16, tag="ea8")
                eb8 = ea8_pool.tile([16, LOCAL], bf16, tag="eb8")
                nc.scalar.activation(ea8[:], F12T[:], AF.Exp)
                nc.scalar.activation(eb8[:], F12T[:], AF.Exp, scale=ALPHA)
                nc.sync.dma_start(EDR[0], ea8[:])
                nc.sync.dma_start(EDR[1], eb8[:])
                for h in range(H):
                    for t in range(2):
                        nc.sync.dma_start(
                            EA4[:, h, t, :],
                            EDR[t, 2 * h : 2 * h + 1, :].partition_broadcast(128),
                        )

                # ---- Phase B: per-head all-gathers (all triggers upfront) ----
                for h in range(H):
                    nc.gpsimd.collective_compute(
                        "AllGather",
                        OP.bypass,
                        replica_groups=[list(range(NCORES))],
                        ins=[LGs[h].opt()],
                        outs=[GGs[l][h].opt()],
                    )

                # ---- Phase C/D: per head: stage gathered slab, then attend ----
                for h in range(H):
                    gsrc = GGs[l][h][:].rearrange("(c p) s -> p c s", p=128)
                    nc.sync.dma_start(
                        GS2[:, :, h * SLOT : (h + 1) * SLOT], gsrc
                    )
                    # f32 copy of r for the STT per-partition scalar
                    nc.scalar.activation(
                        RF2[:, h, :], GS2[:, :, h * SLOT + D + 1], AF.Copy
                    )
                    pb = psB.tile([SLOT - 1, LOCAL], f32, tag="psB")
                    for jc2 in range(0, JC, 2):
                        p2 = p_pool.tile([128, 2 * LOCAL], bf16, tag="p2")
                        for k in range(2):
                            jc = jc2 + k
                            nc.vector.scalar_tensor_tensor(
                                p2[:, k * LOCAL : (k + 1) * LOCAL],
                                EA4[:, h, 1, :],
                                RF2[:, h, jc : jc + 1],
                                EA4[:, h, 0, :],
                                OP.mult,
                                OP.max,
                            )
                        pm2 = pm_pool.tile([128, 2 * LOCAL], bf16, tag="pm2")
                        eng = (
                            nc.vector
                            if (jc2 % 8 == 0 or jc2 == 30)
                            else nc.gpsimd
                        )
                        eng.tensor_tensor(
                            pm2[:],
                            p2[:],
                            MASK[:, jc2 * LOCAL : (jc2 + 2) * LOCAL],
                            OP.mult,
                        )
                        for k in range(2):
                            jc = jc2 + k
                            nc.tensor.matmul(
                                pb[:],
                                GS2[:, jc, h * SLOT : h * SLOT + SLOT - 1],
                                pm2[:, k * LOCAL : (k + 1) * LOCAL],
                                start=(jc == 0),
                                stop=(jc == JC - 1),
                            )

                    # normalize by Z (PSUM row 64) and apply ELU
                    # 1/Z: DMA-reshape Z [1,512] -> [128,4] so the iterative
                    # divide runs 4-deep across 128 lanes, then reshape back.
                    zrow1 = nm_pool.tile([1, LOCAL], f32, tag="zrow1")
                    nc.scalar.copy(zrow1[:], pb[D : D + 1, :])
                    zz4 = nm_pool.tile([128, LOCAL // 128], f32, tag="zz4")
                    nc.sync.dma_start(zz4[:], zrow1[:])
                    rz4 = nm_pool.tile([128, LOCAL // 128], f32, tag="rz4")
                    nc.vector.reciprocal(rz4[:], zz4[:])
                    r1 = nm_pool.tile([1, LOCAL], f32, tag="r1")
                    nc.sync.dma_start(r1[:], rz4[:])
                    rb2 = nm_pool.tile([D, LOCAL], f32, tag="rb2")
                    nc.gpsimd.partition_broadcast(rb2[:], r1[:], channels=D)
                    y = nm_pool.tile([D, LOCAL], f32, tag="y")
                    nc.vector.tensor_tensor(y[:], pb[0:D, :], rb2[:], OP.mult)
                    ee = nm_pool.tile([D, LOCAL], f32, tag="ee")
                    nc.scalar.activation(ee[:], y[:], AF.Exp)
                    ry = nm_pool.tile([D, LOCAL], f32, tag="ry")
                    nc.scalar.activation(ry[:], y[:], AF.Relu)
                    z1 = nm_pool.tile([D, LOCAL], f32, tag="z1")
                    nc.vector.scalar_tensor_tensor(
                        z1[:], ee[:], 1.0, ry[:], OP.min, OP.add
                    )
                    if l < NL - 1:
                        poff = (h % 2) * D
                        dst = X2[poff : poff + D, h // 2, :]
                        nc.scalar.activation(dst, z1[:], AF.Identity, bias=NEG1[0:D, :])
                    else:
                        ey = nm_pool.tile([D, LOCAL], f32, tag="ey")
                        nc.scalar.activation(ey[:], z1[:], AF.Identity, bias=NEG1[0:D, :])
                        if h == 0:
                            nc.vector.tensor_copy(ACCa[:], ey[:])
                        else:
                            src, dst_acc = (
                                (ACCa, ACCb) if h % 2 == 1 else (ACCb, ACCa)
                            )
                            nc.vector.tensor_tensor(
                                dst_acc[:], src[:], ey[:], OP.add
                            )

            # ---- final: mean over heads, ELU, write out ----
            fin = ACCb if (H - 1) % 2 == 1 else ACCa
            m1 = nm_pool.tile([D, LOCAL], f32, tag="m1")
            nc.vector.tensor_scalar(m1[:], fin[:], 1.0 / H, None, OP.mult)
            e2 = nm_pool.tile([D, LOCAL], f32, tag="e2")
            nc.scalar.activation(e2[:], m1[:], AF.Exp)
            r2 = nm_pool.tile([D, LOCAL], f32, tag="r2")
            nc.scalar.activation(r2[:], m1[:], AF.Relu)
            nc.vector.scalar_tensor_tensor(
                OUTS[:], e2[:], 1.0, r2[:], OP.min, OP.add
            )
            nc.vector.tensor_scalar(OUTS[:], OUTS[:], 1.0, None, OP.subtract)
            nc.sync.dma_start(out_d[:], OUTS[:])

    nc.compile()
    return nc


def _prep_inputs(inputs):
    x = np.asarray(inputs["x"], np.float32)
    adj = np.asarray(inputs["adj"])
    Ws = [np.asarray(inputs[k], np.float32) for k in ("W1", "W2", "W3")]
    a1s = [np.asarray(inputs[k], np.float32) for k in ("a1_1", "a1_2", "a1_3")]
    a2s = [np.asarray(inputs[k], np.float32) for k in ("a2_1", "a2_2", "a2_3")]

    wext = np.zeros((NL, F, H * SLOT), np.float32)
    for l in range(NL):
        for h in range(H):
            wext[l, :, h * SLOT : h * SLOT + D] = Ws[l][h]
            wext[l, :, h * SLOT + D] = Ws[l][h] @ a1s[l][h]
            wext[l, :, h * SLOT + D + 1] = Ws[l][h] @ a2s[l][h]
    wext_bf = np.ascontiguousarray(wext.astype(BF))

    mask = adj > 0
    in_maps = []
    for c in range(NCORES):
        rows = slice(c * LOCAL, (c + 1) * LOCAL)
        in_maps.append(
            {
                "xT": np.ascontiguousarray(x[rows].T).astype(BF),
                "maskT": np.ascontiguousarray(mask[rows].T).astype(BF),
                "wext": wext_bf,
            }
        )
    return in_maps


_CACHE = {}


def _run(inputs, trace=False):
    in_maps = _prep_inputs(inputs)
    if "nc" not in _CACHE:
        _CACHE["nc"] = build_nc()
    res = run_bass_kernel_spmd(
        _CACHE["nc"], in_maps, list(range(NCORES)), trace=trace
    )
    outs = [r["outT"] for r in res.results]
    out = np.concatenate([np.asarray(o, np.float32).T for o in outs], axis=0)
    return out, res


def kernel(**inputs) -> np.ndarray:
    out, _ = _run(inputs, trace=False)
    return out.astype(np.float32)
